# revision 26
# baseline (speedup 1.0000x reference)
"""Trainium2 Bass kernel for BioSphericalCKN1D.

out[b,t,f] = scale * dot[b,t,f] / (sqrt(patch_energy[b,t] + 1e-5) + 1e-5) + bias[f]
  dot = conv1d VALID, (B,L,C) x (K,C,F), K=9, C=21, F=128
  patch_energy = sliding sum over the K window and all C channels of x^2

Sharding: data-parallel over batch, 2 batches per core on 8 cores.

Layout trick: slab layout — partition p holds positions [p*256, (p+1)*256)
of one batch (L = 32768 = 128*256). In this layout an im2col patch of 6 taps
(126 values) is a CONTIGUOUS run xn[p, g*21 : g*21+126], so the matmul lhsT
windows come from a single PE transpose of that run — no replication copies.

Per batch:
  1. x is pre-cast to bf16 on the host (halves HBM load traffic) and loaded
     with ONE overlapping-window DMA: partition p reads GW*C contiguous
     elements starting at p*G*C (overlap = halo). Tail overrun is host-padded.
  2. energy: square+reduce -> S [128, 264], sliding-9 doubling adds (free-dim
     shifts only) -> E, sqrt(E+epsE), +epsN, reciprocal -> RC [128, 256] f32
  3. per slab-offset g (window = 128 positions {i*256+g}): PE transpose
     xn[:, g*21 : g*21+128] (128 wide => fast weight load) -> psum ->
     evacuate to RB (engine chosen per EVAC_PAT to balance ACT/DVE)
     mm1: psum  = RB[0:126, g*128 : +128].T @ W1  (taps 0..5)
     mm2: psum += RB[0:63, (g+6)*128 : +128].T @ W2 (taps 6..8; for g >= 250
          the shifted window wraps to column (g-250)*128+1)
     epilogue: out = psum * RC[:, g] on DVE (broadcast tensor_tensor) or ACT
     (activation Copy with per-partition scale), per EPI_PAT; staged in
     groups of G_ST windows per store DMA, all stores on the sync HWDGE ring.

scale is folded into W1/W2 on the host.
"""

import numpy as np

import concourse.bacc as bacc
import concourse.bass as bass
import concourse.tile as tile
from concourse import mybir
from concourse.bass_utils import run_bass_kernel_spmd

F32 = mybir.dt.float32
BF16 = mybir.dt.bfloat16
NP_BF16 = mybir.dt.np(BF16)

B, L, C = 16, 32768, 21
K, F = 9, 128
LOUT = L - K + 1            # 32760
NCORES = 8
BPC = B // NCORES           # 2 batches per core
P = 128
G = L // P                  # 256 positions per slab
HALO = 8                    # K - 1
GW = G + HALO               # 264
S1, S2 = 6, 3               # tap split: mm1 contracts 6*21=126, mm2 3*21=63
R1, R2 = S1 * C, S2 * C     # 126, 63
XPAD = 1024                 # host zero-pad after the last batch (halo overrun)
EPS_ENERGY = 1e-05
EPS_NORM = 1e-05

# schedule knobs
WB = 8                      # windows per build block (one PSUM bank of bf16)
MB = 8                      # windows per matmul group (two PSUM banks f32)
G_ST = 32                   # windows per store DMA (16 KiB runs per row)
LA = 2                      # build blocks of lookahead (mm2 reads g+6)
OUT_BUFS = 5                # store staging buffers (break store->epi WAR chain)
EVAC_PAT = "A"              # evacuation engine per build block (A=ACT, D=DVE)
EPI_PAT = "DDDDDDDDDDDDDDDA"  # epilogue engine per mm group
TR_DMA = False              # transpose via DMA xbar instead of PE (slow on HW)

_COMPILED = {}


def _build(nc, use_bias, reps=1, ablate="full", hw_loop=0,
           evac_pat=EVAC_PAT, epi_pat=EPI_PAT):
    from contextlib import ExitStack
    do_store = ablate in ("full", "indep")
    do_epi = ablate in ("full", "nostore", "indep")
    do_mm = ablate not in ("build_only", "store_only")
    do_compute = ablate != "store_only"
    indep = ablate == "indep"

    x = nc.dram_tensor("x", [BPC * L * C + XPAD], BF16, kind="ExternalInput").ap()
    w1 = nc.dram_tensor("w1", [R1, F], BF16, kind="ExternalInput").ap()
    w2 = nc.dram_tensor("w2", [R2, F], BF16, kind="ExternalInput").ap()
    ident = nc.dram_tensor("ident", [P, P], BF16, kind="ExternalInput").ap()
    biasr = nc.dram_tensor("biasr", [1, F], F32, kind="ExternalInput").ap()
    y = nc.dram_tensor("y", [BPC, LOUT, F], BF16, kind="ExternalOutput").ap()

    with tile.TileContext(nc) as tc, ExitStack() as ctx:
        consts = ctx.enter_context(tc.tile_pool(name="consts", bufs=1))
        big = ctx.enter_context(tc.tile_pool(name="big", bufs=1))
        small = ctx.enter_context(tc.tile_pool(name="small", bufs=2))
        outs = ctx.enter_context(tc.tile_pool(name="outs", bufs=OUT_BUFS))
        pst_pool = ctx.enter_context(tc.tile_pool(name="pst", bufs=2, space="PSUM"))
        pso_pool = ctx.enter_context(tc.tile_pool(name="pso", bufs=3, space="PSUM"))

        w1_sb = consts.tile([R1, F], BF16)
        nc.sync.dma_start(out=w1_sb, in_=w1)
        w2_sb = consts.tile([R2, F], BF16)
        nc.sync.dma_start(out=w2_sb, in_=w2)
        id_sb = consts.tile([P, P], BF16)
        nc.sync.dma_start(out=id_sb, in_=ident)
        eps_sb = consts.tile([P, 1], F32)
        nc.vector.memset(eps_sb, float(EPS_ENERGY))
        bias_sb = None
        if use_bias:
            bias_sb = consts.tile([P, F], F32)
            nc.gpsimd.dma_start(
                out=bias_sb,
                in_=bass.AP(tensor=biasr.tensor, offset=biasr.offset,
                            ap=[[0, P]] + list(biasr.ap[1:])),
            )

        xn_pool = ctx.enter_context(tc.tile_pool(name="xn2", bufs=2))

        dummy_ot = None
        if indep or not do_compute:
            dummy_ot = consts.tile([P, G_ST, F], BF16)
            nc.vector.memset(dummy_ot, 1.0)

        def emit_stores_only(b):
            y_b = y[b]
            for m in range(G // G_ST):
                gs = m * G_ST
                dst = bass.AP(
                    tensor=y_b.tensor, offset=y_b.offset + gs * F,
                    ap=[[G * F, P], [F, G_ST if gs + G_ST < G else G_ST - 8],
                        [1, F]])
                nc.sync.dma_start(
                    out=dst,
                    in_=dummy_ot if gs + G_ST < G
                    else dummy_ot[:, 0:G_ST - 8, :])

        def emit_batch(b):
            if not do_compute:
                emit_stores_only(b)
                return
            xn = xn_pool.tile([P, GW * C], BF16, tag="xn")
            # overlapping-window load: partition p reads [p*G*C, p*G*C + GW*C)
            nc.gpsimd.dma_start(
                out=xn,
                in_=bass.AP(tensor=x.tensor, offset=x.offset + b * L * C,
                            ap=[[G * C, P], [1, GW * C]]),
            )

            # ---- energy path (free-dim shifts only in slab layout) ----
            st = small.tile([P, GW], F32, tag="st")
            xv = xn.rearrange("p (g c) -> p g c", c=C)
            qn = GW // 4
            for h in range(4):
                sq = xn_pool.tile([P, qn, C], BF16, tag="sq")
                sl = slice(h * qn, (h + 1) * qn)
                nc.scalar.activation(
                    out=sq, in_=xv[:, sl, :],
                    func=mybir.ActivationFunctionType.Square)
                nc.vector.tensor_reduce(
                    out=st[:, sl], in_=sq, axis=mybir.AxisListType.X,
                    op=mybir.AluOpType.add)
            t1 = small.tile([P, GW - 1], F32, tag="t1")
            nc.vector.tensor_tensor(
                out=t1, in0=st[:, 0:GW - 1], in1=st[:, 1:GW],
                op=mybir.AluOpType.add)
            t2 = small.tile([P, GW - 3], F32, tag="t2")
            nc.vector.tensor_tensor(
                out=t2, in0=t1[:, 0:GW - 3], in1=t1[:, 2:GW - 1],
                op=mybir.AluOpType.add)
            t4 = small.tile([P, GW - 7], F32, tag="t4")
            nc.vector.tensor_tensor(
                out=t4, in0=t2[:, 0:GW - 7], in1=t2[:, 4:GW - 3],
                op=mybir.AluOpType.add)
            en = small.tile([P, G], F32, tag="en")
            nc.vector.tensor_tensor(
                out=en, in0=t4[:, 0:G], in1=st[:, 8:GW],
                op=mybir.AluOpType.add)

            nre = small.tile([P, G], F32, tag="nre")
            nc.scalar.activation(
                out=nre, in_=en, func=mybir.ActivationFunctionType.Sqrt,
                bias=eps_sb[:, 0:1], scale=1.0)
            ne2 = small.tile([P, G], F32, tag="ne2")
            nc.vector.tensor_scalar_add(ne2, nre, float(EPS_NORM))
            rc = small.tile([P, G], F32, tag="rc")
            nc.vector.reciprocal(out=rc, in_=ne2)

            # ---- window loop ----
            # build blocks of WB windows: WB 128-wide PE transposes into one
            # PSUM bank, one evacuation into RB (engine per EVAC_PAT).
            # Matmul groups of MB windows across two PSUM banks; epilogue per
            # EPI_PAT: DVE broadcast tensor_tensor or ACT per-partition-scale
            # activations.
            rb = big.tile([P, G * P], BF16, tag="rb")
            NBLK = G // WB

            def emit_build_block(q):
                if TR_DMA:
                    # xbar DMA transpose straight into rb: no PE pair, no
                    # PSUM round-trip, no evacuation. Separate HWDGE ring
                    # (scalar) from the stores (sync).
                    for i in range(WB):
                        g = q * WB + i
                        nc.scalar.dma_start_transpose(
                            out=rb[0:P, g * P:(g + 1) * P],
                            in_=xn[:, g * C: g * C + P])
                    return
                pst = pst_pool.tile([P, WB * P], BF16, tag="pstq")
                for i in range(WB):
                    g = q * WB + i
                    nc.tensor.transpose(
                        pst[:, i * P:(i + 1) * P],
                        xn[:, g * C: g * C + P], id_sb)
                # bitcast evacuation: the PSUM bank holds 1024 PACKED bf16
                # per partition; moving them as 512 f32 cells halves the
                # 1-elem/cycle copy cost. Pure bit-move (Copy, scale=1).
                dst = rb[:, q * WB * P:(q + 1) * WB * P].bitcast(F32)
                src = pst.bitcast(F32)
                eng = nc.scalar if evac_pat[q % len(evac_pat)] == "A" \
                    else nc.vector
                if eng is nc.scalar:
                    eng.copy(out=dst, in_=src)
                else:
                    eng.tensor_copy(out=dst, in_=src)

            for q in range(LA):
                emit_build_block(q)

            ot = None
            for m in range(G // MB):
                g0 = m * MB
                for q in range(g0 // WB + LA, (g0 + MB) // WB + LA):
                    if q < NBLK:
                        emit_build_block(q)
                pso = pso_pool.tile([P, MB * F], F32)
                if not do_mm:
                    continue
                # Interleaved emission: all mm1s then all mm2s, so
                # consecutive matmuls hit different PSUM slices and the PE
                # reorder window pulls LDWEIGHTS ahead instead of stalling
                # on each accumulation pair. start=True clears has_written
                # BANK-wide (data survives), so only the first mm touching
                # each 2KB bank sets it; per-element has_written then gives
                # overwrite-on-first-touch / accumulate-on-second.
                for i in range(MB):
                    g = g0 + i
                    nc.tensor.matmul(
                        pso[:, i * F:(i + 1) * F],
                        lhsT=rb[0:R1, g * P: g * P + P], rhs=w1_sb,
                        start=(i % 4 == 0), stop=False)
                for i in range(MB):
                    g = g0 + i
                    off = (g + S1) * P if g + S1 < G else (g + S1 - G) * P + 1
                    nc.tensor.matmul(
                        pso[:, i * F:(i + 1) * F],
                        lhsT=rb[0:R2, off: off + P], rhs=w2_sb,
                        start=False, stop=(i % 4 == 3))
                if not do_epi:
                    continue
                if g0 % G_ST == 0:
                    ot = outs.tile([P, G_ST, F], BF16)
                osl = ot[:, (g0 % G_ST):(g0 % G_ST) + MB, :]
                if epi_pat[m % len(epi_pat)] == "D":
                    rcb = bass.AP(
                        tensor=rc.tensor, offset=rc[:, g0:g0 + 1].offset,
                        ap=[list(rc.ap[0]), [1, MB], [0, F]])
                    nc.vector.tensor_tensor(
                        out=osl, in0=pso.rearrange("p (m f) -> p m f", f=F),
                        in1=rcb, op=mybir.AluOpType.mult)
                else:
                    for i in range(MB):
                        g = g0 + i
                        nc.scalar.activation(
                            out=osl[:, i, :], in_=pso[:, i * F:(i + 1) * F],
                            func=mybir.ActivationFunctionType.Copy,
                            scale=rc[:, g:g + 1], bias=0.0)
                if use_bias:
                    nc.vector.tensor_tensor(
                        out=osl, in0=osl,
                        in1=bass.AP(tensor=bias_sb.tensor,
                                    offset=bias_sb.offset,
                                    ap=[list(bias_sb.ap[0]), [0, MB], [1, F]]),
                        op=mybir.AluOpType.add)
                if (g0 + MB) % G_ST == 0 and do_store:
                    gs = g0 + MB - G_ST
                    src = dummy_ot if indep else ot
                    y_b = y[b]
                    if g0 + MB < G:
                        dst = bass.AP(
                            tensor=y_b.tensor,
                            offset=y_b.offset + gs * F,
                            ap=[[G * F, P], [F, G_ST], [1, F]])
                        nc.sync.dma_start(out=dst, in_=src)
                    else:
                        # final group: partition 127 is valid only through
                        # g = 247 (t <= 32759)
                        NV = G - 8 - gs  # valid windows for partition 127
                        dst1 = bass.AP(
                            tensor=y_b.tensor,
                            offset=y_b.offset + gs * F,
                            ap=[[G * F, P], [F, NV], [1, F]])
                        nc.sync.dma_start(out=dst1, in_=src[:, 0:NV, :])
                        dst2 = bass.AP(
                            tensor=y_b.tensor,
                            offset=y_b.offset + (gs + NV) * F,
                            ap=[[G * F, P - 1], [F, G_ST - NV], [1, F]])
                        nc.sync.dma_start(out=dst2, in_=src[0:P - 1, NV:G_ST, :])

        if hw_loop:
            with tc.For_i(0, hw_loop):
                for b in range(BPC):
                    emit_batch(b)
        else:
            for b in [bb for _ in range(reps) for bb in range(BPC)]:
                emit_batch(b)
    return nc


def _get_program(use_bias, reps=1, ablate="full", hw_loop=0,
                 evac_pat=EVAC_PAT, epi_pat=EPI_PAT):
    key = (bool(use_bias), reps, ablate, hw_loop, evac_pat, epi_pat)
    if key not in _COMPILED:
        nc = bacc.Bacc(
            "TRN2", target_bir_lowering=False, debug=False,
            enable_asserts=False, num_devices=NCORES)
        _build(nc, key[0], reps, ablate, hw_loop, evac_pat, epi_pat)
        nc.compile()
        _COMPILED[key] = nc
    return _COMPILED[key]


def make_in_maps(inp):
    inputs = np.asarray(inp["inputs"], dtype=np.float32)
    kern = np.asarray(inp["kernel"], dtype=np.float32)
    sval = float(np.asarray(inp["scale"]).reshape(-1)[0])
    bias = np.asarray(inp["bias"], dtype=np.float32).reshape(1, F)
    wk = (sval * kern).astype(NP_BF16)  # fold scale into the weights
    w1 = np.ascontiguousarray(wk[0:S1].reshape(R1, F))
    w2 = np.ascontiguousarray(wk[S1:K].reshape(R2, F))
    id128 = np.eye(P, dtype=np.float32).astype(NP_BF16)
    xb = inputs.astype(NP_BF16)  # host cast: halves HBM load traffic
    in_maps = []
    for i in range(NCORES):
        xc = np.zeros(BPC * L * C + XPAD, dtype=NP_BF16)
        xc[:BPC * L * C] = xb[i * BPC:(i + 1) * BPC].reshape(-1)
        in_maps.append({
            "x": xc, "w1": w1, "w2": w2, "ident": id128, "biasr": bias,
        })
    return in_maps


def kernel(inputs, kernel, scale, bias, _trace=False, _trace_kwargs=None,
           _reps=1):
    bias = np.asarray(bias, dtype=np.float32).reshape(1, F)
    use_bias = bool(np.any(bias))
    in_maps = make_in_maps(dict(
        inputs=inputs, kernel=kernel, scale=scale, bias=bias))
    nc = _get_program(use_bias, _reps)
    res = run_bass_kernel_spmd(
        nc, in_maps, list(range(NCORES)), trace=_trace,
        **(_trace_kwargs or {}))
    out = np.concatenate([res.results[i]["y"] for i in range(NCORES)], axis=0)
    if _trace:
        return out.astype(np.float32), res
    return out.astype(np.float32)
